# revision 1
# baseline (speedup 1.0000x reference)
"""Multi-head causal attention block on 8 Trainium2 NeuronCores.

Sharding: tensor-parallel over heads (4 groups of 4 heads) x data-parallel
over batch (2). Core c -> (batch b=c//4, head-group g=c%4). Each core
computes q/k/v projections for its head group, causal attention for its 4
heads, and a partial output projection; the host sums the 4 partials per
batch. All layout transposes are done host-side.

One software-pipelined instruction stream: projections run slice by
slice (512 seq cols); once a slice's k/v/q are resident, that q-block's
attention (whose exp work makes it Activation-bound) is woven, two head
units at a time, tile-by-tile into the next slice's projection matmuls
so the PE never waits on ACT. Output-projection chunks (and slice 3's
deferred v projection) are the filler for the last q-block. Projections
and PV run bf16 (full PE rate at any moving width); the q.k scores run
fp8e4m3 in DoubleRow mode (0.5 PE cycles/row). Softmax denominators
accumulate on DVE + one ones-matmul; output bias bo is added on the
host during the partial-sum gather; the PE runs only real matmul work.

Self-contained: hardcodes shapes for the 2x2048x2048, 16-head problem.
"""

from contextlib import ExitStack

import ml_dtypes
import numpy as np

import concourse.bass as bass
import concourse.tile as tile
from concourse import bacc, mybir
from concourse.bass import ds, ts
from concourse.bass_utils import run_bass_kernel_spmd

F32 = mybir.dt.float32
BF16 = mybir.dt.bfloat16
FP8 = mybir.dt.float8e4
ACTF = mybir.ActivationFunctionType
BFNP = ml_dtypes.bfloat16

# Full-problem dims
BATCH = 2
SEQ = 2048
D_MODEL = 2048
NUM_HEADS = 16
HEAD_DIM = 128
N_CORES = 8
N_GROUPS = 4  # head-groups (tensor parallel)
DG = D_MODEL // N_GROUPS  # 512 = 4 heads per group
SCALE = 1.0 / float(np.sqrt(HEAD_DIM))

QB1 = 512  # projection seq-slice width (512-row matmuls: SEQ-dispatch
#            per PE instruction is ~142ns, so 256-row/107ns matmuls are
#            sequencer-bound; 512-row/213ns are not)
N_SL = SEQ // QB1  # 4 slices == 4 q-blocks
NKD = D_MODEL // 128  # 16 contraction tiles over d_model
QB = 512  # attention q-block width
N_QB = SEQ // QB  # 4 q-blocks
N_DG = DG // 128  # 4 head tiles per group
N_SK = SEQ // 128  # 16 seq tiles


def _mha_body(ctx, tc, aps):
    nc = tc.nc
    S, D, DGl = SEQ, D_MODEL, DG
    xt, wqt, wkt, wvt, wot = (aps[k] for k in ("xt", "wqt", "wkt", "wvt", "wot"))
    out = aps["out"]
    out2 = aps["out2"]

    consts = ctx.enter_context(tc.tile_pool(name="consts", bufs=1))
    ones_sb = consts.tile([128, 1], BF16, name="ones_sb")
    bq_sb = consts.tile([128, N_DG], F32, name="bq_sb")
    bk_sb = consts.tile([128, N_DG], F32, name="bk_sb")
    bv_sb = consts.tile([128, DGl], F32, name="bv_sb")
    tri_sb = consts.tile([128, 128], BF16, name="tri_sb")

    wpool = ctx.enter_context(tc.tile_pool(name="wpool", bufs=1))
    w_sbs = {
        w: wpool.tile([128, NKD * DGl], BF16, name=f"{w}_sb")
        for w in ("wq", "wk", "wv")
    }
    wo_sb = wpool.tile([128, N_DG * D], BF16, name="wo_sb")

    res = ctx.enter_context(tc.tile_pool(name="res", bufs=1))
    # q^T/k^T feed only the scores matmul, which runs fp8e4m3 in DoubleRow
    # mode (0.5 PE cycles/row): packed layout [64 partitions, 2, seq],
    # head-dim hd -> (j=hd//64 plane, p=hd%64). Projections drain to an
    # fp8 staging tile; an SBUF->SBUF DMA repacks partitions 64-127 into
    # plane 1 (engines can't move data across partitions, DMA can).
    kt_res = [res.tile([64, 2 * S], FP8, tag=f"kt{m}", name=f"kt{m}") for m in range(N_DG)]
    qt_res = [res.tile([64, 2 * S], FP8, tag=f"qt{m}", name=f"qt{m}") for m in range(N_DG)]
    v_all = res.tile([128, N_SK * DGl], BF16, name="v_all")  # [p, t*DGl + j]
    ctx_sbs = [res.tile([128, S], BF16, tag=f"cx{m}", name=f"cx{m}") for m in range(N_DG)]

    xpool = ctx.enter_context(tc.tile_pool(name="xpool", bufs=2))
    epool = ctx.enter_context(tc.tile_pool(name="epool", bufs=8))
    accp = ctx.enter_context(tc.tile_pool(name="accp", bufs=2))
    lrec = ctx.enter_context(tc.tile_pool(name="lrec", bufs=2))
    bcp = ctx.enter_context(tc.tile_pool(name="bcp", bufs=2))
    ost = ctx.enter_context(tc.tile_pool(name="ost", bufs=6))
    st8 = ctx.enter_context(tc.tile_pool(name="st8", bufs=8))


    # ---------------- DMA issue helpers ----------------
    def load_w_part(wname, wap, k0, nk):
        nc.sync.dma_start(
            w_sbs[wname][:, ds(k0 * DGl, nk * DGl)].rearrange("p (k f) -> p k f", k=nk),
            wap.rearrange("(k p) f -> p k f", p=128)[:, ds(k0, nk), :],
        )

    def load_wo(k0, nk):
        nc.sync.dma_start(
            wo_sb[:, ds(k0 * D, nk * D)].rearrange("p (k f) -> p k f", k=nk),
            wot.rearrange("(k p) f -> p k f", p=128)[:, ds(k0, nk), :],
        )

    def load_x(s, k_chunks=(8, 8)):
        t = xpool.tile([128, NKD * QB1], BF16, tag="xt", name="xt_sb")
        k0 = 0
        for nk in k_chunks:
            nc.sync.dma_start(
                t[:, ds(k0 * QB1, nk * QB1)].rearrange("p (k f) -> p k f", k=nk),
                xt[ds(k0 * 128, nk * 128), ts(s, QB1)].rearrange(
                    "(k p) f -> p k f", p=128
                ),
            )
            k0 += nk
        return t

    # ---------------- instruction-stream generators ----------------
    # Each generator emits instructions as it is advanced; one `yield` per
    # matmul (or drain) so the weaver can interleave streams finely.


    def drain_pack_qk(dst, ps_ap, b_ap, m, s):
        st = st8.tile([128, QB1], FP8, tag="st8", name="st8")
        nc.scalar.activation(st[:], ps_ap, ACTF.Identity, bias=b_ap)
        # one DMA per plane: a single AP whose partition index strides
        # across the 64/128 boundary reads garbage on hw, so plane j pulls
        # partitions [j*64, j*64+64) with a plain partition-base offset
        for j in range(2):
            nc.sync.dma_start(
                dst[m][:, ds(j * S + s * QB1, QB1)],
                st[ds(j * 64, 64), :],
            )

    def gen_kq(dst, wname, b_sb, s, x_sb, pj):
        # q^T/k^T [head-tile m: 128 hd-dims, QB1 seq] -> resident tiles
        for m in range(N_DG):
            ps = pj.tile([128, QB1], F32, tag="pj", name="ps_kq")
            for k in range(NKD):
                nc.tensor.matmul(
                    ps[:],
                    lhsT=w_sbs[wname][:, ds(k * DGl + m * 128, 128)],
                    rhs=x_sb[:, ts(k, QB1)],
                    start=(k == 0),
                    stop=(k == NKD - 1),
                    skip_group_check=True,
                )
                yield
            drain_pack_qk(dst, ps[:], b_sb[:, ds(m, 1)], m, s)
            yield

    def gen_v(s, x_sb, pj, tag="pj"):
        # v [seq 128, DGl] -> v_all resident
        for msub in range(QB1 // 128):
            ps = pj.tile([128, DGl], F32, tag=tag, name="ps_v")
            for k in range(NKD):
                nc.tensor.matmul(
                    ps[:],
                    lhsT=x_sb[:, ds(k * QB1 + msub * 128, 128)],
                    rhs=w_sbs["wv"][:, ts(k, DGl)],
                    start=(k == 0),
                    stop=(k == NKD - 1),
                    skip_group_check=True,
                )
                yield
            t = (QB1 // 128) * s + msub
            nc.vector.tensor_add(v_all[:, ds(t * DGl, DGl)], ps[:], bv_sb[:])
            yield

    def gen_slice(s, x_sb, pj):
        yield from gen_kq(kt_res, "wk", bk_sb, s, x_sb, pj)
        yield from gen_v(s, x_sb, pj)
        yield from gen_kq(qt_res, "wq", bq_sb, s, x_sb, pj)

    SLICE_STEPS = 3 * 4 * (NKD + 1)  # 204

    def gen_p3(qb, pj3, tag="p3"):
        # output projection for the 4 seq tiles of q-block qb (bo is
        # added host-side). DMA cannot read PSUM, so results stage
        # through ACT/DVE copies before the store; the very last tile
        # uses split stores so the end-of-kernel DMA tail is short.
        for mi in range(4):
            m = qb * 4 + mi
            for n in range(D // QB):
                ps = pj3.tile([128, QB], F32, tag=tag, name="ps_p3")
                for g in range(N_DG):
                    nc.tensor.matmul(
                        ps[:],
                        lhsT=ctx_sbs[g][:, ts(m, 128)],
                        rhs=wo_sb[:, ds(g * D + n * QB, QB)],
                        start=(g == 0),
                        stop=(g == N_DG - 1),
                        skip_group_check=True,
                    )
                    yield
                last = qb == 3 and mi == 3 and n == D // QB - 1
                if qb == 3 and mi >= 2:
                    # the final two seq tiles store bf16 partials into a
                    # side tensor (host converts): halves the serial-DMA
                    # store queue that forms the end-of-kernel tail
                    ot = ost.tile([128, QB], BF16, tag="ot2", name="ot2")
                    if last:
                        for hlf in range(2):
                            nc.scalar.copy(
                                ot[:, ds(hlf * 256, 256)],
                                ps[:, ds(hlf * 256, 256)],
                            )
                            nc.sync.dma_start(
                                out2[
                                    ts(mi - 2, 128),
                                    ds(n * QB + hlf * 256, 256),
                                ],
                                ot[:, ds(hlf * 256, 256)],
                            )
                    else:
                        nc.scalar.copy(ot[:], ps[:])
                        nc.sync.dma_start(
                            out2[ts(mi - 2, 128), ts(n, QB)], ot[:]
                        )
                elif qb == 3:
                    ot = ost.tile([128, QB], F32, tag="ot", name="ot")
                    nc.scalar.copy(ot[:], ps[:])
                    nc.sync.dma_start(out[ts(m, 128), ts(n, QB)], ot[:])
                else:
                    # DMA cannot read PSUM; stage through a copy, split
                    # between ACT and DVE so neither engine saturates in
                    # round 4 (ACT also runs qb3's exp, DVE its softmax).
                    ot = ost.tile([128, QB], F32, tag="ot", name="ot")
                    if (mi * (D // QB) + n) % 4 == 0:
                        nc.scalar.copy(ot[:], ps[:])
                    else:
                        nc.vector.tensor_copy(ot[:], ps[:])
                    nc.sync.dma_start(out[ts(m, 128), ts(n, QB)], ot[:])
                yield

    P3_STEPS = 4 * (D // QB) * (N_DG + 1)  # 80 per q-block

    def attn_unit(h, qb):
        # causal attention for head-tile h over q-block qb; softmax
        # denominator accumulates on DVE (no PE ones-matmul per k-tile);
        # PV for tile t is emitted with scores of tile t+1 so the exp
        # latency is covered by interleaved filler matmuls.
        n_kt = 4 * (qb + 1)
        diag0 = n_kt - 4
        pc = ps_c.tile([128, QB], F32, tag="c", name="ps_c")
        acc = accp.tile([128, QB], BF16, tag="acc", name="acc")

        def emit_pv(kt, sc, w, ex):
            nc.tensor.matmul(
                pc[:, ds(sc, w)],
                lhsT=v_all[:, ds(kt * DGl + h * 128, 128)],
                rhs=ex[:, ds(sc, w)],
                start=(kt == 0),
                stop=(kt == n_kt - 1),
                skip_group_check=True,
            )
            if kt == 0:
                nc.vector.tensor_copy(acc[:], ex[:])
            else:
                nc.vector.tensor_add(
                    acc[:, ds(sc, w)], acc[:, ds(sc, w)], ex[:, ds(sc, w)]
                )

        # software pipeline depth 2: PV for tile t issues two stages after
        # its scores, so the scores->mask->exp chain (~1.3us) is covered
        # even when only ~3 filler matmuls separate stages (round 4).
        pend = []
        for kt in range(n_kt):
            off = kt - diag0
            sc = max(0, off) * 128
            w = QB - sc
            pss = ps_s.tile([128, QB], F32, tag="s", name="ps_s")
            nc.tensor.matmul(
                pss[:, ds(sc, w)],
                lhsT=kt_res[h][:].rearrange("p (j s) -> p j s", j=2)[
                    :, :, ts(kt, 128)
                ],
                rhs=qt_res[h][:].rearrange("p (j s) -> p j s", j=2)[
                    :, :, ds(qb * QB + sc, w)
                ],
                start=True,
                stop=True,
                perf_mode=mybir.MatmulPerfMode.DoubleRow,
            )
            ex = epool.tile([128, QB], BF16, tag="ex", name="ex")
            nc.scalar.activation(ex[:, ds(sc, w)], pss[:, ds(sc, w)], ACTF.Exp, scale=SCALE)
            if off >= 0:
                # zero the masked upper triangle after exp: keeps the
                # scores->exp chain free of any DVE hop (ps_s turnaround
                # gates the 2-deep scores pipeline); PV reads ex two
                # stages later, so this mul is far off the critical path.
                nc.vector.tensor_mul(
                    ex[:, ds(sc, 128)], ex[:, ds(sc, 128)], tri_sb[:]
                )
            pend.append((kt, sc, w, ex))
            if len(pend) > 2:
                emit_pv(*pend.pop(0))
            yield
        for p_ in pend:
            emit_pv(*p_)
        pl = ps_l.tile([1, QB], F32, tag="l", name="ps_l")
        nc.tensor.matmul(pl[:], lhsT=ones_sb[:], rhs=acc[:], start=True, stop=True)
        rec = lrec.tile([1, QB], F32, tag="r", name="rec")
        nc.vector.reciprocal(rec[:], pl[:])
        bc = bcp.tile([128, QB], F32, tag="bc", name="bc")
        nc.gpsimd.partition_broadcast(bc[:], rec[:])
        nc.vector.tensor_mul(ctx_sbs[h][:, ts(qb, QB)], pc[:], bc[:])
        yield

    ATTN_STEPS = lambda qb: 4 * (qb + 1) + 1

    _SENTINEL = object()

    def weave(units, n_unit_steps, filler, n_filler_steps):
        # Bresenham-distribute filler steps across attention unit stages.
        # A burst of fillers at each unit boundary covers the previous
        # unit's still-in-flight exp tiles (ps_s buffer reuse) so the new
        # unit's first scores matmul doesn't stall the PE.
        err = 0
        for iu, u in enumerate(units):
            # no burst before the first unit: at a round boundary the
            # filler's psum banks are still draining from the previous
            # round's last groups (pool-close bank handoff), while the
            # attention ops are independent and can start immediately
            for b in range(3 if iu else 0):
                if next(filler, _SENTINEL) is _SENTINEL:
                    break
                err -= n_unit_steps
            for _ in u:
                err += n_filler_steps
                while err >= n_unit_steps:
                    err -= n_unit_steps
                    if next(filler, _SENTINEL) is _SENTINEL:
                        err = -(1 << 30)
        for _ in filler:
            pass

    # interleave unit pairs: a single unit's exp demand (~610ns/tile on
    # ACT) outruns its own PE ops (~320ns/tile), and the OOO window
    # pre-spends fillers, starving late stages. Two units woven
    # tile-by-tile double the per-stage PE attention work so ACT stays
    # just under the stage period.
    def pair(ua, ub):
        its = [ua, ub]
        done = [False, False]
        while not all(done):
            for i_ in range(2):
                if not done[i_]:
                    if next(its[i_], _SENTINEL) is _SENTINEL:
                        done[i_] = True
                    else:
                        yield

    # ---------------- top-level schedule ----------------
    # The DMA device is serial in the sim (~1.46us per 4KB/line chunk), so
    # arrival order must track PE consumption order: a sliver of wk to
    # start, bias consts early (psum drains need them!), x slice 0, the
    # rest of wk, then wv / wq / x slice 1. A warmup matmul block finishes
    # the PE clock ramp while the first loads are in flight.
    # interleave wk parts and x0 chunks in PE-consumption order: the
    # first projection group's k0-3 matmuls can start once wk k0-3 and
    # x0 k0-3 land (~5.8us into the serial DMA stream)
    load_w_part("wk", wkt, 0, 4)
    xa = load_x(0, k_chunks=(4,))
    load_w_part("wk", wkt, 4, 6)
    _x0_c2 = nc.sync.dma_start(
        xa[:, ds(4 * QB1, 4 * QB1)].rearrange("p (k f) -> p k f", k=4),
        xt[ds(4 * 128, 4 * 128), ts(0, QB1)].rearrange("(k p) f -> p k f", p=128),
    )
    load_w_part("wk", wkt, 10, 6)
    nc.sync.dma_start(
        xa[:, ds(8 * QB1, 4 * QB1)].rearrange("p (k f) -> p k f", k=4),
        xt[ds(8 * 128, 4 * 128), ts(0, QB1)].rearrange("(k p) f -> p k f", p=128),
    )
    nc.sync.dma_start(
        xa[:, ds(12 * QB1, 4 * QB1)].rearrange("p (k f) -> p k f", k=4),
        xt[ds(12 * 128, 4 * 128), ts(0, QB1)].rearrange("(k p) f -> p k f", p=128),
    )
    nc.sync.dma_start(bk_sb[:], aps["bk"])
    nc.sync.dma_start(bq_sb[:], aps["bq"])
    nc.sync.dma_start(bv_sb[:], aps["bv"])
    nc.sync.dma_start(tri_sb[:], aps["tri"])
    nc.sync.dma_start(ones_sb[:], aps["ones"])
    load_w_part("wv", wvt, 0, 8)
    load_w_part("wv", wvt, 8, 8)
    load_w_part("wq", wqt, 0, 8)
    load_w_part("wq", wqt, 8, 8)
    xb = load_x(1)

    # warmup: the serial DMA device needs ~13us to deliver wk + x slice 0,
    # so run dummy matmuls until then. This both hides the DMA preamble
    # and finishes the PE clock ramp (3us of continuous use) so the real
    # stream starts at full speed with no gap (any PE idle gap resets the
    # ramp and costs ~1us of mid-p-state time).
    with tc.tile_pool(name="warm", bufs=1) as wrm, tc.tile_pool(
        name="warm_ps", bufs=1, space="PSUM"
    ) as wps:
        wtile = wrm.tile([128, 512], BF16, name="warm_sb")
        nc.gpsimd.memset(wtile[:], 0.0)
        wp = wps.tile([128, 512], F32, tag="w", name="warm_ps")
        NWARM = 10
        for i in range(NWARM):
            nc.tensor.matmul(
                wp[:],
                lhsT=wtile[:, ds(0, 128)],
                rhs=wtile[:],
                start=(i == 0),
                stop=(i == NWARM - 1),
            )

    # round 0: slice-0 projections in a dedicated psum scope (attention
    # pools don't exist yet, so four banks hold all head-tiles' groups
    # open across a k-split: the PE starts on the first half of x slice 0
    # while the second half is still in flight on the serial DMA device).
    with (
        tc.tile_pool(name="pj0a", bufs=1, space="PSUM") as pj0a,
        tc.tile_pool(name="pj0b", bufs=2, space="PSUM") as pj0b,
    ):

        def kq0(dst, wname, b_sb):
            pss = [
                pj0a.tile([128, QB1], F32, tag=f"p0{m}", name="ps_kq0")
                for m in range(N_DG)
            ]
            for kh in (0, 1):
                for m in range(N_DG):
                    for k in range(8 * kh, 8 * kh + 8):
                        nc.tensor.matmul(
                            pss[m][:],
                            lhsT=w_sbs[wname][:, ds(k * DGl + m * 128, 128)],
                            rhs=xa[:, ts(k, QB1)],
                            start=(k == 0),
                            stop=(k == NKD - 1),
                            skip_group_check=True,
                        )
            for m in range(N_DG):
                drain_pack_qk(dst, pss[m][:], b_sb[:, ds(m, 1)], m, 0)

        kq0(kt_res, "wk", bk_sb)
        for _ in gen_v(0, xa, pj0b):
            pass
        kq0(qt_res, "wq", bq_sb)

    ps_s = ctx.enter_context(tc.tile_pool(name="ps_s", bufs=2, space="PSUM"))
    ps_c = ctx.enter_context(tc.tile_pool(name="ps_c", bufs=2, space="PSUM"))
    ps_l = ctx.enter_context(tc.tile_pool(name="ps_l", bufs=1, space="PSUM"))

    # one 3-deep projection/output psum ring lives from round 1 through
    # round 5: ring continuity avoids the pool-close bank-handoff WARs
    # that stalled the round 3 -> 4 transition (the freed banks were
    # still draining when the next pool's first groups allocated them).
    with tc.tile_pool(name="pj", bufs=3, space="PSUM") as pj:
        # rounds 1-3: slice P woven with attention over q-block P-1
        for P in (1, 2, 3):
            xa = xb
            xb = load_x(P + 1) if P < 3 else None
            if P == 2:
                load_wo(0, 2)
                load_wo(2, 2)
            qb = P - 1
            units = [
                pair(attn_unit(0, qb), attn_unit(1, qb)),
                pair(attn_unit(2, qb), attn_unit(3, qb)),
            ]
            if P < 3:
                filler, fsteps = gen_slice(P, xa, pj), SLICE_STEPS
            else:
                # defer slice 3's v projection into round 4: its 13.6us of
                # PE work is only needed by the last PV tiles of q-block 3,
                # and round 4 is otherwise thin on PE filler relative to
                # its exp (ACT) load.
                def kq_only(s, x_sb):
                    yield from gen_kq(kt_res, "wk", bk_sb, s, x_sb, pj)
                    yield from gen_kq(qt_res, "wq", bq_sb, s, x_sb, pj)

                filler, fsteps = kq_only(P, xa), 2 * 4 * (NKD + 1)
                xa3 = xa
            weave(units, N_DG * ATTN_STEPS(qb), filler, max(1, fsteps - 28))

        # round 4: last q-block's attention woven with output projection
        # of q-blocks 0-2; then the remaining output projection.
        units = [
            pair(attn_unit(0, 3), attn_unit(1, 3)),
            pair(attn_unit(2, 3), attn_unit(3, 3)),
        ]

        def p3_fill():
            yield from gen_v(3, xa3, pj, tag="pj")
            for qb in range(3):
                yield from gen_p3(qb, pj, tag="pj")

        weave(
            units,
            N_DG * ATTN_STEPS(3),
            p3_fill(),
            4 * (NKD + 1) + 3 * P3_STEPS - 28,
        )
        for _ in gen_p3(3, pj, tag="pj"):
            pass


def build_program(enable_asserts=False):
    nc = bacc.Bacc(
        "TRN2",
        target_bir_lowering=False,
        debug=False,
        enable_asserts=enable_asserts,
        num_devices=N_CORES,
    )
    S, D, DGl = SEQ, D_MODEL, DG
    aps = {
        "xt": nc.dram_tensor("xt", [D, S], BF16, kind="ExternalInput").ap(),
        "wqt": nc.dram_tensor("wqt", [D, DGl], BF16, kind="ExternalInput").ap(),
        "wkt": nc.dram_tensor("wkt", [D, DGl], BF16, kind="ExternalInput").ap(),
        "wvt": nc.dram_tensor("wvt", [D, DGl], BF16, kind="ExternalInput").ap(),
        "wot": nc.dram_tensor("wot", [DGl, D], BF16, kind="ExternalInput").ap(),
        "bq": nc.dram_tensor("bq", [128, DGl // 128], F32, kind="ExternalInput").ap(),
        "bk": nc.dram_tensor("bk", [128, DGl // 128], F32, kind="ExternalInput").ap(),
        "bv": nc.dram_tensor("bv", [128, DGl], F32, kind="ExternalInput").ap(),
        "tri": nc.dram_tensor("tri", [128, 128], BF16, kind="ExternalInput").ap(),
        "ones": nc.dram_tensor("ones", [128, 1], BF16, kind="ExternalInput").ap(),
        "out": nc.dram_tensor("out", [S, D], F32, kind="ExternalOutput").ap(),
        "out2": nc.dram_tensor("out2", [256, D], BF16, kind="ExternalOutput").ap(),
    }
    with tile.TileContext(nc) as tc:
        with ExitStack() as ctx:
            _mha_body(ctx, tc, aps)
    nc.compile()
    return nc


def make_tri():
    """Multiplicative causal mask for the 128x128 diagonal block: 1 where
    kpos<=qpos (keep), 0 where masked (applied to exp'd scores)."""
    p = np.arange(128)[:, None]
    f = np.arange(128)[None, :]
    return np.where(p <= f, 1.0, 0.0).astype(BFNP)


def shard_inputs(x, wq, bq, wk, bk, wv, bv, wo, bo):
    """Build the 8 per-core input maps (host-side layout prep + bf16)."""
    tri = make_tri()
    xts = [
        np.ascontiguousarray(np.asarray(x[b], np.float32).T).astype(BFNP)
        for b in range(BATCH)
    ]
    in_maps = []
    for c in range(N_CORES):
        b, g = divmod(c, N_GROUPS)
        sl = slice(g * DG, (g + 1) * DG)
        in_maps.append(
            {
                "xt": xts[b],
                "wqt": np.ascontiguousarray(np.asarray(wq, np.float32)[sl].T).astype(BFNP),
                "wkt": np.ascontiguousarray(np.asarray(wk, np.float32)[sl].T).astype(BFNP),
                "wvt": np.ascontiguousarray(np.asarray(wv, np.float32)[sl].T).astype(BFNP),
                "wot": np.ascontiguousarray(np.asarray(wo, np.float32)[:, sl].T).astype(BFNP),
                "bq": np.ascontiguousarray(
                    np.asarray(bq, np.float32)[sl].reshape(-1, 128).T
                ),
                "bk": np.ascontiguousarray(
                    np.asarray(bk, np.float32)[sl].reshape(-1, 128).T
                ),
                "bv": np.ascontiguousarray(
                    np.broadcast_to(np.asarray(bv, np.float32)[sl], (128, DG))
                ),
                "tri": tri,
                "ones": np.ones((128, 1), BFNP),
            }
        )
    return in_maps


_NC_CACHE = {}


def get_program():
    if "nc" not in _NC_CACHE:
        _NC_CACHE["nc"] = build_program()
    return _NC_CACHE["nc"]


def run_sharded(inputs, trace=False):
    nc = get_program()
    in_maps = shard_inputs(**inputs)
    res = run_bass_kernel_spmd(nc, in_maps, list(range(N_CORES)), trace=trace)
    bo = np.asarray(inputs["bo"], np.float32)
    full = np.empty((BATCH, SEQ, D_MODEL), np.float32)
    for b in range(BATCH):
        acc = res.results[b * N_GROUPS]["out"].copy()
        acc2 = res.results[b * N_GROUPS]["out2"].astype(np.float32)
        for g in range(1, N_GROUPS):
            acc += res.results[b * N_GROUPS + g]["out"]
            acc2 += res.results[b * N_GROUPS + g]["out2"].astype(np.float32)
        acc[SEQ - 256 :] = acc2
        full[b] = acc + bo
    return full, res


def kernel(**inputs):
    out, _ = run_sharded(inputs, trace=False)
    return out



# revision 23
# speedup vs baseline: 1.4141x; 1.4141x over previous
"""Multi-head causal attention block on 8 Trainium2 NeuronCores.

Sharding: tensor-parallel over heads (4 groups of 4 heads) x data-parallel
over batch (2). Core c -> (batch b=c//4, head-group g=c%4). Each core
computes q/k/v projections for its head group, causal attention for its 4
heads, and a partial output projection; the host sums the 4 partials per
batch (plus bo and the v-bias term bv@wo^T, which is exact host algebra
since sum_k p_k = 1).

All heavy matmuls run fp8e4m3 in DoubleRow mode (0.5 PE-cycles/row). The
two DoubleRow planes are spent either on contraction pairing (256-wide
contraction = 4x bf16 throughput) or on (hi,lo) residual compensation:
a tensor T is stored as T_hi + T_lo, two fp8 values at the SAME power-of-2
scale, recovering ~bf16 accuracy while keeping fp8 speed. Which tensors
get compensated was chosen empirically against the 2e-2 gate:
  - x: hi/lo (lo used only in the v-projection; q/k tolerate x_hi alone)
  - wv, wo: hi/lo (host-side, free); wq/wk single
  - k-drain: hi/lo woven into the scores planes (k_hi+k_lo)@q_hi: free
  - v-drain, ctx: hi/lo (two PV / out-proj passes)
  - exp(probs): single fp8 (softmax denominator uses the same quantized
    values, cancelling the systematic part)
Predicted rel-err 1.6e-2 (sim_precision3.py); PE ~152us.

Softmax denominators come from a 0.25-valued ones DoubleRow matmul over
the fp8 prob pairs (rec = 4/l feeds the ctx scale 2^6 for free). hi/lo
drains go through a bf16 staging tile: full = psum-scale-op, hi = copy,
lo = full - hi; no extra PE work and one psum read per tile.

Self-contained: hardcodes shapes for the 2x2048x2048, 16-head problem.
"""

from contextlib import ExitStack

import ml_dtypes
import numpy as np

import concourse.bass as bass
import concourse.tile as tile
from concourse import bacc, mybir
from concourse.bass import ds, ts
from concourse.bass_utils import run_bass_kernel_spmd

F32 = mybir.dt.float32
BF16 = mybir.dt.bfloat16
FP8 = mybir.dt.float8e4
ACTF = mybir.ActivationFunctionType
ALU = mybir.AluOpType
BFNP = ml_dtypes.bfloat16
E4NP = ml_dtypes.float8_e4m3

# Full-problem dims
BATCH = 2
SEQ = 2048
D_MODEL = 2048
NUM_HEADS = 16
HEAD_DIM = 128
N_CORES = 8
N_GROUPS = 4  # head-groups (tensor parallel)
DG = D_MODEL // N_GROUPS  # 512 = 4 heads per group
SCALE = 1.0 / float(np.sqrt(HEAD_DIM))

QB1 = 512  # projection seq-slice width
N_SL = SEQ // QB1  # 4 slices
NKD = D_MODEL // 128  # 16 contraction tiles over d_model
NPR = NKD // 2  # 8 contraction pairs
QB = 512  # attention q-block width
N_QB = SEQ // QB  # 4 q-blocks
N_DG = DG // 128  # 4 head tiles per group
N_SK = SEQ // 128  # 16 seq tiles

# power-of-2 scales (exponents)
XS = 4  # x stored at 2^4
WS = 5  # weights stored at 2^5
QKS = 4  # q/k drained at 2^4
VS = 4  # v drained at 2^4
CS = 6  # ctx stored at 2^6 (via ones=0.25 -> rec = 4/l)
DR_P = 2.0 ** (QKS - XS - WS)  # proj psum -> drain scale 2^-5
EXPS = SCALE * 2.0 ** (-2 * QKS)  # exp scale on scores psum
OUTS = 2.0 ** (-CS - WS)  # out-proj psum -> bf16 scale 2^-11


def _mha_body(ctx, tc, aps):
    nc = tc.nc
    S, D, DGl = SEQ, D_MODEL, DG
    out = aps["out"]

    consts = ctx.enter_context(tc.tile_pool(name="consts", bufs=1))
    onesf = consts.tile([128, 256], FP8, name="onesf")
    bq4 = consts.tile([128, N_DG], F32, name="bq4")
    bk4 = consts.tile([128, N_DG], F32, name="bk4")
    tri_sb = consts.tile([128, 128], FP8, name="tri_sb")

    wpool = ctx.enter_context(tc.tile_pool(name="wpool", bufs=1))
    wq_sb = wpool.tile([128, NKD * DGl], FP8, name="wq_sb")
    wk_sb = wpool.tile([128, NKD * DGl], FP8, name="wk_sb")
    wvhd_sb = wpool.tile([128, NKD * 2 * DGl], FP8, name="wvhd_sb")
    wvl_sb = wpool.tile([128, NKD * DGl], FP8, name="wvl_sb")
    wohd_sb = wpool.tile([128, N_DG * 2 * D], FP8, name="wohd_sb")
    wol_sb = wpool.tile([128, N_DG * D], FP8, name="wol_sb")

    res = ctx.enter_context(tc.tile_pool(name="res", bufs=1))
    # kt[m]: [128 hd, (hi S | lo S)]; qt[m]: [128 hd, (hi S | hi-dup S)]
    kt_res = [res.tile([128, 2 * S], FP8, tag=f"kt{m}", name=f"kt{m}") for m in range(N_DG)]
    qt_res = [res.tile([128, 2 * S], FP8, tag=f"qt{m}", name=f"qt{m}") for m in range(N_DG)]
    v_hi = res.tile([128, N_SK * DGl], FP8, name="v_hi")  # [p, t*DGl + hd]
    v_lo = res.tile([128, N_SK * DGl], FP8, name="v_lo")
    # ctx: [128 hd, (g, hi/lo, S)]
    ctx_sb = res.tile([128, N_DG * 2 * S], FP8, name="ctx_sb")

    xpool = ctx.enter_context(tc.tile_pool(name="xpool", bufs=2))
    epool = ctx.enter_context(tc.tile_pool(name="epool", bufs=6))
    stg = ctx.enter_context(tc.tile_pool(name="stg", bufs=6))
    cstg = ctx.enter_context(tc.tile_pool(name="cstg", bufs=2))
    lrec = ctx.enter_context(tc.tile_pool(name="lrec", bufs=2))
    ost = ctx.enter_context(tc.tile_pool(name="ost", bufs=6))

    # ---------------- DMA issue helpers ----------------
    def load_w(dst, wap, k0, nk, width):
        # dst free layout [(k, width)]; src rows (k p), cols width
        nc.sync.dma_start(
            dst[:, ds(k0 * width, nk * width)].rearrange("p (k f) -> p k f", k=nk),
            wap.rearrange("(k p) f -> p k f", p=128)[:, ds(k0, nk), :],
        )

    def load_x_plane(t, s, j, k0=0, nk=NKD):
        # xhl tile free layout [(k, j, f)]; src xhl [D, 2S]: cols j*S + s*QB1
        nc.sync.dma_start(
            t.rearrange("p (k b) -> p k b", k=NKD)[:, ds(k0, nk), ds(j * QB1, QB1)],
            aps["xhl"].rearrange("(k p) f -> p k f", p=128)[
                :, ds(k0, nk), ds(j * S + s * QB1, QB1)
            ],
        )

    # ---------------- hi/lo drain helper ----------------
    def drain_hl(ps_ap, hi_ap, lo_ap, scale, bias=None, eng="v"):
        full = stg.tile([128, QB1], BF16, tag="stg", name="stg")
        if bias is None:
            nc.vector.tensor_scalar_mul(full[:], ps_ap, scale)
        else:
            nc.vector.tensor_scalar(full[:], ps_ap, scale, bias, ALU.mult, ALU.add)
        nc.vector.tensor_copy(hi_ap, full[:])
        nc.vector.tensor_tensor(lo_ap, full[:], hi_ap, ALU.subtract)

    # ---------------- instruction-stream generators ----------------
    def gen_qk(dst, w_sb, b_sb, s, x_sb, pj, dup):
        # q^T/k^T [head-tile m: 128 hd, QB1 seq]
        for m in range(N_DG):
            ps = pj.tile([128, QB1], F32, tag="pj", name="ps_kq")
            for u in range(NPR):
                nc.tensor.matmul(
                    ps[:],
                    lhsT=w_sb[:, ds(2 * u * DGl, 2 * DGl)].rearrange(
                        "p (k f) -> p k f", k=2
                    )[:, :, ds(m * 128, 128)],
                    rhs=x_sb.rearrange("p (k b) -> p k b", k=NKD)[
                        :, ds(2 * u, 2), ds(0, QB1)
                    ],
                    start=(u == 0),
                    stop=(u == NPR - 1),
                    perf_mode=mybir.MatmulPerfMode.DoubleRow,
                    skip_group_check=True,
                )
                yield
            if dup:  # q: hi drain + plane-dup via DMA
                nc.vector.tensor_scalar(
                    dst[m][:, ds(s * QB1, QB1)], ps[:], DR_P, b_sb[:, ds(m, 1)],
                    ALU.mult, ALU.add,
                )
                nc.sync.dma_start(
                    dst[m][:, ds(S + s * QB1, QB1)], dst[m][:, ds(s * QB1, QB1)]
                )
            else:  # k: hi/lo drains
                drain_hl(
                    ps[:],
                    dst[m][:, ds(s * QB1, QB1)],
                    dst[m][:, ds(S + s * QB1, QB1)],
                    DR_P,
                    bias=b_sb[:, ds(m, 1)],
                )
            yield

    def gen_v(s, x_sb, pj):
        # v [seq 128, DGl] -> v_hi/v_lo resident
        for msub in range(QB1 // 128):
            ps = pj.tile([128, DGl], F32, tag="pj", name="ps_v")
            # set1: per k-tile, planes = (x_hi, x_lo) x (wv_hi, wv_hi)
            for k in range(NKD):
                nc.tensor.matmul(
                    ps[:],
                    lhsT=x_sb[:, ds(k * 2 * QB1, 2 * QB1)].rearrange(
                        "p (j f) -> p j f", j=2
                    )[:, :, ds(msub * 128, 128)],
                    rhs=wvhd_sb[:, ds(k * 2 * DGl, 2 * DGl)].rearrange(
                        "p (j f) -> p j f", j=2
                    ),
                    start=(k == 0),
                    stop=False,
                    perf_mode=mybir.MatmulPerfMode.DoubleRow,
                    skip_group_check=True,
                )
                yield
            # set2: k-paired planes, x_hi x wv_lo
            for u in range(NPR):
                nc.tensor.matmul(
                    ps[:],
                    lhsT=x_sb.rearrange("p (k b) -> p k b", k=NKD)[
                        :, ds(2 * u, 2), ds(msub * 128, 128)
                    ],
                    rhs=wvl_sb[:, ds(2 * u * DGl, 2 * DGl)].rearrange(
                        "p (k f) -> p k f", k=2
                    ),
                    start=False,
                    stop=(u == NPR - 1),
                    perf_mode=mybir.MatmulPerfMode.DoubleRow,
                    skip_group_check=True,
                )
                yield
            t = (QB1 // 128) * s + msub
            drain_hl(
                ps[:], v_hi[:, ds(t * DGl, DGl)], v_lo[:, ds(t * DGl, DGl)],
                2.0 ** (VS - XS - WS),
            )
            yield

    def gen_slice(s, x_sb, pj):
        yield from gen_qk(kt_res, wk_sb, bk4, s, x_sb, pj, dup=False)
        yield from gen_v(s, x_sb, pj)
        yield from gen_qk(qt_res, wq_sb, bq4, s, x_sb, pj, dup=True)

    SLICE_STEPS = 2 * 4 * (NPR + 1) + 4 * (NKD + NPR + 1)  # 172

    def gen_p3(qb, pj):
        # output projection for the 4 seq tiles of q-block qb
        for mi in range(4):
            m = qb * 4 + mi
            for n in range(D // QB):
                ps = pj.tile([128, QB], F32, tag="pj", name="ps_p3")
                # set1: per g, planes = (ctx_hi, ctx_lo) x (wo_hi, wo_hi)
                for g in range(N_DG):
                    nc.tensor.matmul(
                        ps[:],
                        lhsT=ctx_sb[:, ds(g * 2 * S, 2 * S)].rearrange(
                            "p (j s) -> p j s", j=2
                        )[:, :, ts(m, 128)],
                        rhs=wohd_sb[:, ds(g * 2 * D, 2 * D)].rearrange(
                            "p (j f) -> p j f", j=2
                        )[:, :, ds(n * QB, QB)],
                        start=(g == 0),
                        stop=False,
                        perf_mode=mybir.MatmulPerfMode.DoubleRow,
                        skip_group_check=True,
                    )
                    yield
                # set2: g-paired planes, ctx_hi x wo_lo
                for gp in range(N_DG // 2):
                    nc.tensor.matmul(
                        ps[:],
                        lhsT=ctx_sb.rearrange("p (g b) -> p g b", g=N_DG)[
                            :, ds(2 * gp, 2), ds(m * 128, 128)
                        ],
                        rhs=wol_sb[:, ds(2 * gp * D, 2 * D)].rearrange(
                            "p (g f) -> p g f", g=2
                        )[:, :, ds(n * QB, QB)],
                        start=False,
                        stop=(gp == N_DG // 2 - 1),
                        perf_mode=mybir.MatmulPerfMode.DoubleRow,
                        skip_group_check=True,
                    )
                    yield
                ot = ost.tile([128, QB], BF16, tag="ot", name="ot")
                # spread psum->bf16 drains across ACT and DVE
                if (mi * (D // QB) + n) % 2 == 0:
                    nc.scalar.mul(ot[:], ps[:], OUTS)
                else:
                    nc.vector.tensor_scalar_mul(ot[:], ps[:], OUTS)
                nc.sync.dma_start(out[ts(m, 128), ts(n, QB)], ot[:])
                yield

    P3_STEPS = 4 * (D // QB) * (N_DG + N_DG // 2 + 1)  # 112 per q-block

    def attn_unit(h, qb):
        # causal attention for head-tile h over q-block qb. The denominator
        # matmul uses a full-rank 0.25-ones lhsT so pl comes out already
        # partition-broadcast: pl[p, q] = l[q]/4 for every p.
        n_kt = 4 * (qb + 1)
        diag0 = n_kt - 4
        n_pr = n_kt // 2
        pc = ps_c.tile([128, QB], F32, tag="c", name="ps_c")
        pl = ps_l.tile([128, QB], F32, tag="l", name="ps_l")

        def emit_pair(u, sc_p, w_p, ex):
            ex_ap = ex.rearrange("p (j f) -> p j f", j=2)[:, :, ds(sc_p, w_p)]
            for vi, v_t in enumerate((v_hi, v_lo)):
                nc.tensor.matmul(
                    pc[:, ds(sc_p, w_p)],
                    lhsT=v_t.rearrange("p (t f) -> p t f", t=N_SK)[
                        :, ds(2 * u, 2), ds(h * 128, 128)
                    ],
                    rhs=ex_ap,
                    start=(u == 0 and vi == 0),
                    stop=(u == n_pr - 1 and vi == 1),
                    perf_mode=mybir.MatmulPerfMode.DoubleRow,
                    skip_group_check=True,
                )
            nc.tensor.matmul(
                pl[:, ds(sc_p, w_p)],
                lhsT=onesf.rearrange("p (j f) -> p j f", j=2),
                rhs=ex_ap,
                start=(u == 0),
                stop=(u == n_pr - 1),
                perf_mode=mybir.MatmulPerfMode.DoubleRow,
                skip_group_check=True,
            )

        pend = []
        for u in range(n_pr):
            ex = epool.tile([128, 2 * QB], FP8, tag="ex", name="ex")
            sc_p = max(0, 2 * u - diag0) * 128
            for j in range(2):
                t = 2 * u + j
                off = t - diag0
                sc = max(0, off) * 128
                w = QB - sc
                pss = ps_s.tile([128, QB], F32, tag="s", name="ps_s")
                nc.tensor.matmul(
                    pss[:, ds(sc, w)],
                    lhsT=kt_res[h].rearrange("p (j s) -> p j s", j=2)[
                        :, :, ts(t, 128)
                    ],
                    rhs=qt_res[h].rearrange("p (j s) -> p j s", j=2)[
                        :, :, ds(qb * QB + sc, w)
                    ],
                    start=True,
                    stop=True,
                    perf_mode=mybir.MatmulPerfMode.DoubleRow,
                )
                yield
                if j == 1 and sc > sc_p:
                    # diagonal pair: zero plane1's fully-masked gap
                    nc.gpsimd.memset(ex[:, ds(QB + sc_p, sc - sc_p)], 0.0)
                nc.scalar.activation(
                    ex[:, ds(j * QB + sc, w)], pss[:, ds(sc, w)], ACTF.Exp, scale=EXPS
                )
                if off >= 0:
                    # zero the masked upper triangle of the diag block
                    nc.gpsimd.tensor_mul(
                        ex[:, ds(j * QB + sc, 128)], ex[:, ds(j * QB + sc, 128)],
                        tri_sb[:],
                    )
                yield
            pend.append((u, sc_p, w_p_of(u, diag0), ex))
            if len(pend) > 1:
                emit_pair(*pend.pop(0))
                yield
        for p_ in pend:
            emit_pair(*p_)
            yield
        rec = lrec.tile([128, QB], F32, tag="r", name="rec")
        nc.vector.reciprocal(rec[:], pl[:])
        cxn = cstg.tile([128, QB], BF16, tag="cx", name="cxn")
        nc.vector.tensor_mul(cxn[:], pc[:], rec[:])
        hi_ap = ctx_sb[:, ds(h * 2 * S + qb * QB, QB)]
        nc.vector.tensor_copy(hi_ap, cxn[:])
        nc.vector.tensor_tensor(
            ctx_sb[:, ds(h * 2 * S + S + qb * QB, QB)], cxn[:], hi_ap, ALU.subtract
        )
        yield

    def w_p_of(u, diag0):
        return QB - max(0, 2 * u - diag0) * 128

    ATTN_STEPS = lambda qb: 3 * (2 * (qb + 1)) + 2  # yields per unit (approx)

    _SENTINEL = object()

    def weave(units, n_unit_steps, filler, n_filler_steps):
        # Bresenham-distribute filler steps across attention unit stages.
        err = 0
        for iu, u in enumerate(units):
            for b in range(3 if iu else 0):
                if next(filler, _SENTINEL) is _SENTINEL:
                    break
                err -= n_unit_steps
            for _ in u:
                err += n_filler_steps
                while err >= n_unit_steps:
                    err -= n_unit_steps
                    if next(filler, _SENTINEL) is _SENTINEL:
                        err = -(1 << 30)
        for _ in filler:
            pass

    def pair(ua, ub):
        its = [ua, ub]
        done = [False, False]
        while not all(done):
            for i_ in range(2):
                if not done[i_]:
                    if next(its[i_], _SENTINEL) is _SENTINEL:
                        done[i_] = True
                    else:
                        yield

    # ---------------- top-level schedule ----------------
    # Serial DMA device: arrival order tracks PE consumption order.
    load_w(wk_sb, aps["wk8"], 0, NKD, DGl)
    xa = xpool.tile([128, NKD * 2 * QB1], FP8, tag="xt", name="xt_sb")
    load_x_plane(xa, 0, 0)  # slice0 hi
    nc.sync.dma_start(bk4[:], aps["bk4"])
    nc.sync.dma_start(bq4[:], aps["bq4"])
    nc.sync.dma_start(tri_sb[:], aps["tri"])
    nc.sync.dma_start(onesf[:], aps["onesf"])
    load_w(wvhd_sb, aps["wvhd"], 0, NKD, 2 * DGl)
    load_w(wvl_sb, aps["wvl"], 0, NKD, DGl)
    load_x_plane(xa, 0, 1)  # slice0 lo
    load_w(wq_sb, aps["wq8"], 0, NKD, DGl)
    xb = xpool.tile([128, NKD * 2 * QB1], FP8, tag="xt", name="xt_sb")
    load_x_plane(xb, 1, 0)
    load_x_plane(xb, 1, 1)
    load_w(wohd_sb, aps["wohd"], 0, N_DG, 2 * D)
    load_w(wol_sb, aps["wol"], 0, N_DG, D)

    # psum pools: 3 + 3 + 1 + 1 = 8 banks
    pj = ctx.enter_context(tc.tile_pool(name="pj", bufs=3, space="PSUM"))
    ps_s = ctx.enter_context(tc.tile_pool(name="ps_s", bufs=3, space="PSUM"))
    ps_c = ctx.enter_context(tc.tile_pool(name="ps_c", bufs=1, space="PSUM"))
    ps_l = ctx.enter_context(tc.tile_pool(name="ps_l", bufs=1, space="PSUM"))

    # warmup: cover the DMA preamble and finish the PE clock ramp
    with tc.tile_pool(name="warm", bufs=1) as wrm:
        wtile = wrm.tile([128, 512], BF16, name="warm_sb")
        nc.gpsimd.memset(wtile[:], 0.0)
        NWARM = 13
        for i in range(NWARM):
            wp = pj.tile([128, 512], F32, tag="pj", name="warm_ps")
            nc.tensor.matmul(
                wp[:], lhsT=wtile[:, ds(0, 128)], rhs=wtile[:],
                start=True, stop=True, skip_group_check=True,
            )

    # round 0: slice-0 projections straight
    for _ in gen_slice(0, xa, pj):
        pass

    # rounds 1-3: slice P projections woven with attention over q-block P-1
    for P in (1, 2, 3):
        xa = xb
        xb = None
        if P < 3:
            xb = xpool.tile([128, NKD * 2 * QB1], FP8, tag="xt", name="xt_sb")
            load_x_plane(xb, P + 1, 0)
            load_x_plane(xb, P + 1, 1)
        qb = P - 1
        units = [attn_unit(h, qb) for h in range(N_DG)]
        if P < 3:
            filler, fsteps = gen_slice(P, xa, pj), SLICE_STEPS
        else:
            def slice3_p30(x_sb):
                yield from gen_slice(3, x_sb, pj)
                yield from gen_p3(0, pj)

            filler, fsteps = slice3_p30(xa), SLICE_STEPS + P3_STEPS
        weave(units, 4 * ATTN_STEPS(qb), filler, max(1, fsteps - 20))

    # round 4: last q-block's attention woven with output projection 1-2
    units = [attn_unit(h, 3) for h in range(N_DG)]

    def p3_12():
        yield from gen_p3(1, pj)
        yield from gen_p3(2, pj)

    weave(units, 4 * ATTN_STEPS(3), p3_12(), 2 * P3_STEPS - 20)
    for _ in gen_p3(3, pj):
        pass


def build_program(enable_asserts=False):
    nc = bacc.Bacc(
        "TRN2",
        target_bir_lowering=False,
        debug=False,
        enable_asserts=enable_asserts,
        num_devices=N_CORES,
    )
    S, D, DGl = SEQ, D_MODEL, DG
    aps = {
        "xhl": nc.dram_tensor("xhl", [D, 2 * S], FP8, kind="ExternalInput").ap(),
        "wq8": nc.dram_tensor("wq8", [D, DGl], FP8, kind="ExternalInput").ap(),
        "wk8": nc.dram_tensor("wk8", [D, DGl], FP8, kind="ExternalInput").ap(),
        "wvhd": nc.dram_tensor("wvhd", [D, 2 * DGl], FP8, kind="ExternalInput").ap(),
        "wvl": nc.dram_tensor("wvl", [D, DGl], FP8, kind="ExternalInput").ap(),
        "wohd": nc.dram_tensor("wohd", [DGl, 2 * D], FP8, kind="ExternalInput").ap(),
        "wol": nc.dram_tensor("wol", [DGl, D], FP8, kind="ExternalInput").ap(),
        "bq4": nc.dram_tensor("bq4", [128, N_DG], F32, kind="ExternalInput").ap(),
        "bk4": nc.dram_tensor("bk4", [128, N_DG], F32, kind="ExternalInput").ap(),
        "tri": nc.dram_tensor("tri", [128, 128], FP8, kind="ExternalInput").ap(),
        "onesf": nc.dram_tensor("onesf", [128, 256], FP8, kind="ExternalInput").ap(),
        "out": nc.dram_tensor("out", [S, D], BF16, kind="ExternalOutput").ap(),
    }
    with tile.TileContext(nc) as tc:
        with ExitStack() as ctx:
            _mha_body(ctx, tc, aps)
    nc.compile()
    return nc


def make_tri():
    """Multiplicative causal mask for the 128x128 diagonal block: 1 where
    kpos<=qpos (keep), 0 where masked (applied to exp'd scores)."""
    p = np.arange(128)[:, None]
    f = np.arange(128)[None, :]
    return np.where(p <= f, 1.0, 0.0).astype(E4NP)


def _hl(a, s):
    """Split a (f32) into fp8 (hi, lo) stored at scale 2^s (same scale)."""
    sc = np.float32(2.0**s)
    hi = (a * sc).astype(E4NP)
    lo = ((a * sc) - hi.astype(np.float32)).astype(E4NP)
    return hi, lo


def shard_inputs(x, wq, bq, wk, bk, wv, bv, wo, bo):
    """Build the 8 per-core input maps (host-side layout prep + fp8)."""
    tri = make_tri()
    onesf = np.full((128, 256), 0.25, E4NP)
    xhls = []
    for b in range(BATCH):
        xT = np.ascontiguousarray(np.asarray(x[b], np.float32).T)
        hi, lo = _hl(xT, XS)
        xhls.append(np.ascontiguousarray(np.concatenate([hi, lo], axis=1)))
    in_maps = []
    for c in range(N_CORES):
        b, g = divmod(c, N_GROUPS)
        sl = slice(g * DG, (g + 1) * DG)
        wqT = np.ascontiguousarray(np.asarray(wq, np.float32)[sl].T)
        wkT = np.ascontiguousarray(np.asarray(wk, np.float32)[sl].T)
        wvT = np.ascontiguousarray(np.asarray(wv, np.float32)[sl].T)
        woT = np.ascontiguousarray(np.asarray(wo, np.float32)[:, sl].T)
        wvh, wvl_ = _hl(wvT, WS)
        woh, wol_ = _hl(woT, WS)
        in_maps.append(
            {
                "xhl": xhls[b],
                "wq8": (wqT * np.float32(2.0**WS)).astype(E4NP),
                "wk8": (wkT * np.float32(2.0**WS)).astype(E4NP),
                "wvhd": np.ascontiguousarray(np.concatenate([wvh, wvh], axis=1)),
                "wvl": wvl_,
                "wohd": np.ascontiguousarray(np.concatenate([woh, woh], axis=1)),
                "wol": wol_,
                "bq4": np.ascontiguousarray(
                    (np.asarray(bq, np.float32)[sl] * 2.0**QKS)
                    .reshape(-1, 128).T.astype(np.float32)
                ),
                "bk4": np.ascontiguousarray(
                    (np.asarray(bk, np.float32)[sl] * 2.0**QKS)
                    .reshape(-1, 128).T.astype(np.float32)
                ),
                "tri": tri,
                "onesf": onesf,
            }
        )
    return in_maps


_NC_CACHE = {}


def get_program():
    if "nc" not in _NC_CACHE:
        _NC_CACHE["nc"] = build_program()
    return _NC_CACHE["nc"]


def run_sharded(inputs, trace=False):
    nc = get_program()
    in_maps = shard_inputs(**inputs)
    res = run_bass_kernel_spmd(nc, in_maps, list(range(N_CORES)), trace=trace)
    bo = np.asarray(inputs["bo"], np.float64)
    bv = np.asarray(inputs["bv"], np.float64)
    wo = np.asarray(inputs["wo"], np.float64)
    bias = (bo + bv @ wo.T).astype(np.float32)
    full = np.empty((BATCH, SEQ, D_MODEL), np.float32)
    for b in range(BATCH):
        acc = res.results[b * N_GROUPS]["out"].astype(np.float32)
        for g in range(1, N_GROUPS):
            acc += res.results[b * N_GROUPS + g]["out"].astype(np.float32)
        full[b] = acc + bias
    return full, res


def kernel(**inputs):
    out, _ = run_sharded(inputs, trace=False)
    return out
